# revision 55
# baseline (speedup 1.0000x reference)
"""Trainium2 Bass kernel for batched Gaussian log-density quadratic form.

Computes out = -einsum('nd,de,ne->n', Y, prec, Y) with Y = X - mean,
X: [65536, 256] f32, mean: [1, 256] f32, prec: [256, 256] f32.

Strategy (data-parallel over rows, 8 NeuronCores):
  Algebraic rewrite: with u = (P + P^T) m and c = m^T P m,
      y^T P y = x^T P x - x.u + c
  so with the augmented moving operand P' = [P | -u] and X~ = [X | 1]:
      sum_e (X @ P')[i,e] * X~[i,e]  =  x^T P x - x.u  =  y^T P y - c
  Per 128-row tile:
    - DMA rows in (batched 4 tiles / 512KB per DMA)
    - PE transposes X tile (2x 128x128, via identity) -> PSUM
    - ACT copies X^T PSUM->SBUF (stationary operand for the matmul)
    - 2 accumulating fp32r matmuls: Z~ = X @ P'  (PSUM, [128, 257])
    - one DVE tensor_tensor_reduce: accum = -c + sum(-(Z~ * X~)) = -y^T P y
      written straight into a staging column.
  Final: PE-transpose staging [128, 64] -> [64, 128], ACT copy, 1 output DMA.
"""

import os
import numpy as np

N, D = 65536, 256
N_CORES = 8
NS = N // N_CORES  # 8192 rows per core
P = 128
TILES = NS // P  # 64 tiles per core
DMA_BATCH = 4  # row-tiles per input DMA (512KB transfers)
DP1 = D + 2  # 258: [P | -u | 0]; fp32r matmul needs an even free dim

TRACE = False
LAST_EXEC_NS = None
LAST_RESULTS = None

_PROGRAM = None


def _build_program():
    import concourse.bass as bass
    import concourse.tile as tile
    from concourse import bacc, mybir
    from contextlib import ExitStack

    F32 = mybir.dt.float32
    F32R = mybir.dt.float32r
    MULT = mybir.AluOpType.mult
    ADD = mybir.AluOpType.add

    nc = bacc.Bacc("TRN2", target_bir_lowering=False, debug=False)
    # host pads X with a ones column (and one zero col for fp32r evenness)
    # and pre-rounds to fp32r so the PE transposes can run in fp32r mode
    x_dram = nc.dram_tensor("x", [NS, DP1], F32R, kind="ExternalInput").ap()
    # p[p, k, :] = [prec | -u | 0][128*k + p, :], host pre-rounded to fp32r
    p_dram = nc.dram_tensor("p", [P, 2, DP1], F32R, kind="ExternalInput").ap()
    negc_dram = nc.dram_tensor("negc", [P, 1], F32, kind="ExternalInput").ap()
    ident_dram = nc.dram_tensor("ident", [P, P], F32R, kind="ExternalInput").ap()
    out_dram = nc.dram_tensor("out", [NS], F32, kind="ExternalOutput").ap()

    with tile.TileContext(nc) as tc, ExitStack() as ctx:
        singles = ctx.enter_context(tc.tile_pool(name="singles", bufs=1))
        xpool = ctx.enter_context(tc.tile_pool(name="xpool", bufs=6))
        xtpool = ctx.enter_context(tc.tile_pool(name="xtpool", bufs=3))
        wpool = ctx.enter_context(tc.tile_pool(name="wpool", bufs=3))
        psum_xt = ctx.enter_context(tc.tile_pool(name="psum_xt", bufs=2, space="PSUM"))
        psum_z = ctx.enter_context(tc.tile_pool(name="psum_z", bufs=2, space="PSUM"))

        # small preamble loads on the ACT HWDGE ring so they don't serialize
        # ahead of the first X loads on the SP ring
        ident = singles.tile([P, P], F32R)
        nc.scalar.dma_start(ident, ident_dram)
        pp = singles.tile([P, 2, DP1], F32R)
        nc.scalar.dma_start(pp, p_dram)
        negc = singles.tile([P, 1], F32)
        nc.scalar.dma_start(negc, negc_dram)
        # warm the ACT function table (~2.7us) off the critical path
        act_warm = singles.tile([P, 1], F32)
        nc.scalar.activation(
            act_warm,
            negc,
            mybir.ActivationFunctionType.Copy,
            scale=1.0,
            accum_out=None,
        )

        # two half-staging tiles so the mid-kernel flush of half 0 creates no
        # WAR dependency against the second half's reduce writes
        staging0 = singles.tile([P, TILES // 2], F32)
        staging1 = singles.tile([P, TILES // 2], F32)
        stagings = [staging0, staging1]

        def stage_col(t):
            h, off = divmod(t, TILES // 2)
            return stagings[h][:, off : off + 1]

        x_view = x_dram.rearrange("(t p) d -> p t d", p=P)  # [128, 64, 258]
        out_view = out_dram.rearrange("(t p) -> t p", p=P)
        H = TILES // 2

        def flush_half(h):
            # out[128*t + p] = staging[p, t]: transpose then contiguous DMA.
            # The final copy adds -c (reduces produced -sum = c - y^T P y).
            # borrows an xt_ps slot (same tag) — saves a PSUM bank
            st_ps = psum_xt.tile([H, P], F32, tag="xt_ps")
            nc.tensor.transpose(st_ps, stagings[h], ident.bitcast(F32))
            out_sb = singles.tile([H, P], F32, tag=f"out_sb{h}")
            nc.scalar.activation(
                out_sb,
                st_ps,
                mybir.ActivationFunctionType.Identity,
                bias=negc[0:H, 0:1],
                scale=1.0,
            )
            nc.sync.dma_start(out_view[h * H : (h + 1) * H, :], out_sb)

        for g in range(TILES // DMA_BATCH):
            if g == 0:
                # split the first group into pair DMAs so compute starts
                # after 256KB instead of 512KB (trims the startup stall)
                xg0 = []
                for j in range(2):
                    xb = xpool.tile([P, 2, DP1], F32R, tag="xg0")
                    nc.sync.dma_start(xb, x_view[:, 2 * j : 2 * j + 2, :])
                    xg0.append(xb)
                xpair = lambda j: xg0[j]
            else:
                xg = xpool.tile([P, DMA_BATCH, DP1], F32R)
                nc.sync.dma_start(
                    xg, x_view[:, g * DMA_BATCH : (g + 1) * DMA_BATCH, :]
                )
                xpair = lambda j: xg[:, 2 * j : 2 * j + 2, :]
            # transpose all 4 row-tiles into one 2-bank PSUM tile, then one
            # wide ACT copy (FD=1024) amortizes the ~370-cycle fixed overhead
            xt_ps = psum_xt.tile([P, 2 * DMA_BATCH, P], F32R)
            for b in range(DMA_BATCH):
                xr = xpair(b // 2)[:, b % 2, :]
                nc.tensor.transpose(xt_ps[:, 2 * b, :], xr[:, 0:P], ident)
                nc.tensor.transpose(xt_ps[:, 2 * b + 1, :], xr[:, P:D], ident)
            xt_sb = xtpool.tile([P, 2 * DMA_BATCH, P], F32R)
            if g == 0:
                # two half-copies so the first matmuls start after 2 tiles
                nc.scalar.copy(xt_sb[:, 0:4, :], xt_ps[:, 0:4, :])
                nc.scalar.copy(xt_sb[:, 4:8, :], xt_ps[:, 4:8, :])
            else:
                nc.scalar.copy(xt_sb, xt_ps)
            for j in range(2):  # process row-tiles in pairs
                pair = 2 * g + j
                t = 2 * pair
                # two tiles' Z side by side in one 2-bank PSUM tile
                z2 = psum_z.tile([P, 2, 512], F32)
                for b2 in range(2):
                    for k in range(2):
                        nc.tensor.matmul(
                            z2[:, b2, 0:DP1],
                            lhsT=xt_sb[:, 2 * (2 * j + b2) + k, :],
                            rhs=pp[:, k, :],
                            start=(k == 0),
                            stop=(k == 1),
                        )
                # W = Z~ * X~ for the pair in ONE DVE op (FD=516); reduces
                # split DVE/ACT to balance (fused reduce ops crash this rt)
                w2 = wpool.tile([P, 2, DP1], F32)
                nc.vector.tensor_mul(
                    w2, z2[:, :, 0:DP1], xpair(j).bitcast(F32)
                )
                if pair % 8 >= 3:  # ACT takes the first 3 of each 8-block
                    h, off = divmod(t, H)
                    nc.vector.tensor_reduce(
                        stagings[h][:, off : off + 2],
                        w2,
                        axis=mybir.AxisListType.X,
                        op=ADD,
                        negate=True,
                    )
                else:
                    for b2 in range(2):
                        nc.scalar.activation(
                            w2[:, b2, :],
                            w2[:, b2, :],
                            mybir.ActivationFunctionType.Copy,
                            scale=-1.0,
                            accum_out=stage_col(t + b2),
                        )
                if pair == TILES // 4 + 2:
                    # staging0 completed 2 pairs ago → the flush transpose
                    # enters the in-order PE queue with no pending wait
                    flush_half(0)

        flush_half(1)

    nc.compile()

    return nc


def _get_program():
    global _PROGRAM
    if _PROGRAM is None:
        _PROGRAM = _build_program()
    return _PROGRAM


def _host_inputs(X, mean, prec):
    X = np.asarray(X, dtype=np.float32)
    X_pad = np.empty((N, DP1), dtype=np.float32)
    # pre-round X to fp32r (11-bit mantissa): fp32r-mode PE reads truncate
    xb = X.view(np.uint32)
    X_pad[:, :D].view(np.uint32)[:] = (xb + 0x800) & np.uint32(0xFFFFF000)
    X_pad[:, D] = 1.0
    X_pad[:, D + 1] = 0.0
    m = np.asarray(mean, dtype=np.float32).reshape(-1)
    Pm = np.asarray(prec, dtype=np.float32)
    u = (Pm + Pm.T) @ m
    c = float(m @ (Pm @ m))
    p_aug = np.concatenate(
        [Pm, -u[:, None], np.zeros((D, 1), np.float32)], axis=1
    )  # [256, 258]
    # pre-round to fp32r (fp32 with 11-bit mantissa, round-half-up on 12 LSBs)
    bits = p_aug.view(np.uint32)
    p_aug = (((bits + 0x800) & np.uint32(0xFFFFF000)).astype(np.uint32)).view(
        np.float32
    )
    p_host = np.ascontiguousarray(
        p_aug.reshape(2, P, DP1).transpose(1, 0, 2)
    )  # [128, 2, 258]
    negc_host = np.full((P, 1), -c, dtype=np.float32)
    ident_host = np.eye(P, dtype=np.float32)
    in_maps = [
        {
            "x": X_pad[i * NS : (i + 1) * NS],
            "p": p_host,
            "negc": negc_host,
            "ident": ident_host,
        }
        for i in range(N_CORES)
    ]
    return in_maps


def kernel(X, mean, prec):
    global LAST_EXEC_NS, LAST_RESULTS
    from concourse.bass_utils import run_bass_kernel_spmd

    nc = _get_program()
    in_maps = _host_inputs(X, mean, prec)
    res = run_bass_kernel_spmd(
        nc, in_maps, core_ids=list(range(N_CORES)), trace=TRACE
    )
    LAST_RESULTS = res
    LAST_EXEC_NS = res.exec_time_ns
    out = np.concatenate([res.results[i]["out"] for i in range(N_CORES)])
    return out.astype(np.float32)


# revision 60
# speedup vs baseline: 1.0189x; 1.0189x over previous
"""Trainium2 Bass kernel for batched Gaussian log-density quadratic form.

Computes out = -einsum('nd,de,ne->n', Y, prec, Y) with Y = X - mean,
X: [65536, 256] f32, mean: [1, 256] f32, prec: [256, 256] f32.

Strategy (data-parallel over rows, 8 NeuronCores):
  Algebraic rewrite: with u = (P + P^T) m and c = m^T P m,
      y^T P y = x^T P x - x.u + c
  so with the augmented moving operand P' = [P | -u] and X~ = [X | 1]:
      sum_e (X @ P')[i,e] * X~[i,e]  =  x^T P x - x.u  =  y^T P y - c
  Per 128-row tile:
    - DMA rows in (batched 4 tiles / 512KB per DMA)
    - PE transposes X tile (2x 128x128, via identity) -> PSUM
    - ACT copies X^T PSUM->SBUF (stationary operand for the matmul)
    - 2 accumulating fp32r matmuls: Z~ = X @ P'  (PSUM, [128, 257])
    - one DVE tensor_tensor_reduce: accum = -c + sum(-(Z~ * X~)) = -y^T P y
      written straight into a staging column.
  Final: PE-transpose staging [128, 64] -> [64, 128], ACT copy, 1 output DMA.
"""

import os
import numpy as np

N, D = 65536, 256
N_CORES = 8
NS = N // N_CORES  # 8192 rows per core
P = 128
TILES = NS // P  # 64 tiles per core
DMA_BATCH = 4  # row-tiles per input DMA (512KB transfers)
DP1 = D + 2  # 258: [P | -u | 0]; fp32r matmul needs an even free dim

TRACE = False
LAST_EXEC_NS = None
LAST_RESULTS = None

_PROGRAM = None


def _build_program():
    import concourse.bass as bass
    import concourse.tile as tile
    from concourse import bacc, mybir
    from contextlib import ExitStack

    F32 = mybir.dt.float32
    F32R = mybir.dt.float32r
    MULT = mybir.AluOpType.mult
    ADD = mybir.AluOpType.add

    nc = bacc.Bacc("TRN2", target_bir_lowering=False, debug=False)
    # host pads X with a ones column (and one zero col for fp32r evenness)
    # and pre-rounds to fp32r so the PE transposes can run in fp32r mode
    x_dram = nc.dram_tensor("x", [NS, DP1], F32R, kind="ExternalInput").ap()
    # p[p, k, :] = [prec | -u | 0][128*k + p, :], host pre-rounded to fp32r
    p_dram = nc.dram_tensor("p", [P, 2, DP1], F32R, kind="ExternalInput").ap()
    negc_dram = nc.dram_tensor("negc", [P, 1], F32, kind="ExternalInput").ap()
    ident_dram = nc.dram_tensor("ident", [P, P], F32R, kind="ExternalInput").ap()
    out_dram = nc.dram_tensor("out", [NS], F32, kind="ExternalOutput").ap()

    with tile.TileContext(nc) as tc, ExitStack() as ctx:
        singles = ctx.enter_context(tc.tile_pool(name="singles", bufs=1))
        xpool = ctx.enter_context(tc.tile_pool(name="xpool", bufs=6))
        xtpool = ctx.enter_context(tc.tile_pool(name="xtpool", bufs=4))
        wpool = ctx.enter_context(tc.tile_pool(name="wpool", bufs=4))
        psum_xt = ctx.enter_context(tc.tile_pool(name="psum_xt", bufs=2, space="PSUM"))
        psum_z = ctx.enter_context(tc.tile_pool(name="psum_z", bufs=2, space="PSUM"))

        # small preamble loads on the ACT HWDGE ring so they don't serialize
        # ahead of the first X loads on the SP ring
        ident = singles.tile([P, P], F32R)
        nc.scalar.dma_start(ident, ident_dram)
        pp = singles.tile([P, 2, DP1], F32R)
        nc.scalar.dma_start(pp, p_dram)
        negc = singles.tile([P, 1], F32)
        nc.scalar.dma_start(negc, negc_dram)
        # warm the ACT function table (~2.7us) off the critical path
        act_warm = singles.tile([P, 1], F32)
        nc.scalar.activation(
            act_warm,
            negc,
            mybir.ActivationFunctionType.Copy,
            scale=1.0,
            accum_out=None,
        )

        # two half-staging tiles so the mid-kernel flush of half 0 creates no
        # WAR dependency against the second half's reduce writes
        staging0 = singles.tile([P, TILES // 2], F32)
        staging1 = singles.tile([P, TILES // 2], F32)
        stagings = [staging0, staging1]

        def stage_col(t):
            h, off = divmod(t, TILES // 2)
            return stagings[h][:, off : off + 1]

        x_view = x_dram.rearrange("(t p) d -> p t d", p=P)  # [128, 64, 258]
        out_view = out_dram.rearrange("(t p) -> t p", p=P)
        H = TILES // 2

        def flush_half(h):
            # out[128*t + p] = staging[p, t]: transpose then contiguous DMA.
            # The final copy adds -c (reduces produced -sum = c - y^T P y).
            # borrows an xt_ps slot (same tag) — saves a PSUM bank
            st_ps = psum_xt.tile([H, P], F32, tag="xt_ps")
            nc.tensor.transpose(st_ps, stagings[h], ident.bitcast(F32))
            out_sb = singles.tile([H, P], F32, tag=f"out_sb{h}")
            # NOTE: must stay on ACT — DVE tensor_scalar with an AP scalar
            # lowers to InstTensorScalarPtr, which crashes this runtime
            nc.scalar.activation(
                out_sb,
                st_ps,
                mybir.ActivationFunctionType.Identity,
                bias=negc[0:H, 0:1],
                scale=1.0,
            )
            nc.sync.dma_start(out_view[h * H : (h + 1) * H, :], out_sb)

        for g in range(TILES // DMA_BATCH):
            if g == 0:
                # split the first group into pair DMAs so compute starts
                # after 256KB instead of 512KB (trims the startup stall)
                xg0 = []
                for j in range(2):
                    xb = xpool.tile([P, 2, DP1], F32R, tag="xg0")
                    nc.sync.dma_start(xb, x_view[:, 2 * j : 2 * j + 2, :])
                    xg0.append(xb)
                xpair = lambda j: xg0[j]
            else:
                xg = xpool.tile([P, DMA_BATCH, DP1], F32R)
                nc.sync.dma_start(
                    xg, x_view[:, g * DMA_BATCH : (g + 1) * DMA_BATCH, :]
                )
                xpair = lambda j: xg[:, 2 * j : 2 * j + 2, :]
            # transpose all 4 row-tiles into one 2-bank PSUM tile, then one
            # wide ACT copy (FD=1024) amortizes the ~370-cycle fixed overhead
            xt_ps = psum_xt.tile([P, 2 * DMA_BATCH, P], F32R)
            for b in range(DMA_BATCH):
                xr = xpair(b // 2)[:, b % 2, :]
                nc.tensor.transpose(xt_ps[:, 2 * b, :], xr[:, 0:P], ident)
                nc.tensor.transpose(xt_ps[:, 2 * b + 1, :], xr[:, P:D], ident)
            xt_sb = xtpool.tile([P, 2 * DMA_BATCH, P], F32R)
            if g == 0:
                # two half-copies so the first matmuls start after 2 tiles
                nc.scalar.copy(xt_sb[:, 0:4, :], xt_ps[:, 0:4, :])
                nc.scalar.copy(xt_sb[:, 4:8, :], xt_ps[:, 4:8, :])
            else:
                nc.scalar.copy(xt_sb, xt_ps)
            for j in range(2):  # process row-tiles in pairs
                pair = 2 * g + j
                t = 2 * pair
                # two tiles' Z side by side in one 2-bank PSUM tile
                z2 = psum_z.tile([P, 2, 512], F32)
                for b2 in range(2):
                    for k in range(2):
                        nc.tensor.matmul(
                            z2[:, b2, 0:DP1],
                            lhsT=xt_sb[:, 2 * (2 * j + b2) + k, :],
                            rhs=pp[:, k, :],
                            start=(k == 0),
                            stop=(k == 1),
                        )
                # W = Z~ * X~ for the pair in ONE DVE op (FD=516); reduces
                # split DVE/ACT to balance (fused reduce ops crash this rt)
                w2 = wpool.tile([P, 2, DP1], F32)
                nc.vector.tensor_mul(
                    w2, z2[:, :, 0:DP1], xpair(j).bitcast(F32)
                )
                if pair % 8 >= 3:  # ACT takes the first 3 of each 8-block
                    h, off = divmod(t, H)
                    nc.vector.tensor_reduce(
                        stagings[h][:, off : off + 2],
                        w2,
                        axis=mybir.AxisListType.X,
                        op=ADD,
                        negate=True,
                    )
                else:
                    for b2 in range(2):
                        nc.scalar.activation(
                            w2[:, b2, :],
                            w2[:, b2, :],
                            mybir.ActivationFunctionType.Copy,
                            scale=-1.0,
                            accum_out=stage_col(t + b2),
                        )
                if pair == TILES // 4 + 2:
                    # staging0 completed 2 pairs ago → the flush transpose
                    # enters the in-order PE queue with no pending wait
                    flush_half(0)

        flush_half(1)

    nc.compile()

    return nc


def _get_program():
    global _PROGRAM
    if _PROGRAM is None:
        _PROGRAM = _build_program()
    return _PROGRAM


def _host_inputs(X, mean, prec):
    X = np.asarray(X, dtype=np.float32)
    X_pad = np.empty((N, DP1), dtype=np.float32)
    # pre-round X to fp32r (11-bit mantissa): fp32r-mode PE reads truncate
    xb = X.view(np.uint32)
    X_pad[:, :D].view(np.uint32)[:] = (xb + 0x800) & np.uint32(0xFFFFF000)
    X_pad[:, D] = 1.0
    X_pad[:, D + 1] = 0.0
    m = np.asarray(mean, dtype=np.float32).reshape(-1)
    Pm = np.asarray(prec, dtype=np.float32)
    u = (Pm + Pm.T) @ m
    c = float(m @ (Pm @ m))
    p_aug = np.concatenate(
        [Pm, -u[:, None], np.zeros((D, 1), np.float32)], axis=1
    )  # [256, 258]
    # pre-round to fp32r (fp32 with 11-bit mantissa, round-half-up on 12 LSBs)
    bits = p_aug.view(np.uint32)
    p_aug = (((bits + 0x800) & np.uint32(0xFFFFF000)).astype(np.uint32)).view(
        np.float32
    )
    p_host = np.ascontiguousarray(
        p_aug.reshape(2, P, DP1).transpose(1, 0, 2)
    )  # [128, 2, 258]
    negc_host = np.full((P, 1), -c, dtype=np.float32)
    ident_host = np.eye(P, dtype=np.float32)
    in_maps = [
        {
            "x": X_pad[i * NS : (i + 1) * NS],
            "p": p_host,
            "negc": negc_host,
            "ident": ident_host,
        }
        for i in range(N_CORES)
    ]
    return in_maps


def kernel(X, mean, prec):
    global LAST_EXEC_NS, LAST_RESULTS
    from concourse.bass_utils import run_bass_kernel_spmd

    nc = _get_program()
    in_maps = _host_inputs(X, mean, prec)
    res = run_bass_kernel_spmd(
        nc, in_maps, core_ids=list(range(N_CORES)), trace=TRACE
    )
    LAST_RESULTS = res
    LAST_EXEC_NS = res.exec_time_ns
    out = np.concatenate([res.results[i]["out"] for i in range(N_CORES)])
    return out.astype(np.float32)


# revision 61
# speedup vs baseline: 1.1284x; 1.1074x over previous
"""Trainium2 Bass kernel for batched Gaussian log-density quadratic form.

Computes out = -einsum('nd,de,ne->n', Y, prec, Y) with Y = X - mean,
X: [65536, 256] f32, mean: [1, 256] f32, prec: [256, 256] f32.

Strategy (data-parallel over rows, 8 NeuronCores):
  Algebraic rewrite: with u = (P + P^T) m and c = m^T P m,
      y^T P y = x^T P x - x.u + c
  so with the augmented moving operand P' = [P | -u] and X~ = [X | 1]:
      sum_e (X @ P')[i,e] * X~[i,e]  =  x^T P x - x.u  =  y^T P y - c
  Per 128-row tile:
    - DMA rows in (batched 4 tiles / 512KB per DMA)
    - PE transposes X tile (2x 128x128, via identity) -> PSUM
    - ACT copies X^T PSUM->SBUF (stationary operand for the matmul)
    - 2 accumulating fp32r matmuls: Z~ = X @ P'  (PSUM, [128, 257])
    - one DVE tensor_tensor_reduce: accum = -c + sum(-(Z~ * X~)) = -y^T P y
      written straight into a staging column.
  Final: PE-transpose staging [128, 64] -> [64, 128], ACT copy, 1 output DMA.
"""

import os
import numpy as np

N, D = 65536, 256
N_CORES = 8
NS = N // N_CORES  # 8192 rows per core
P = 128
TILES = NS // P  # 64 tiles per core
DMA_BATCH = 4  # row-tiles per input DMA (512KB transfers)
DP1 = D + 2  # 258: [P | -u | 0]; fp32r matmul needs an even free dim

TRACE = False
LAST_EXEC_NS = None
LAST_RESULTS = None

_PROGRAM = None


def _build_program():
    import concourse.bass as bass
    import concourse.tile as tile
    from concourse import bacc, mybir
    from contextlib import ExitStack

    F32 = mybir.dt.float32
    F32R = mybir.dt.float32r
    MULT = mybir.AluOpType.mult
    ADD = mybir.AluOpType.add

    nc = bacc.Bacc("TRN2", target_bir_lowering=False, debug=False)
    # host pads X with a ones column (and one zero col for fp32r evenness)
    # and pre-rounds to fp32r so the PE transposes can run in fp32r mode
    x_dram = nc.dram_tensor("x", [NS, DP1], F32R, kind="ExternalInput").ap()
    # p[p, k, :] = [prec | -u | 0][128*k + p, :], host pre-rounded to fp32r
    p_dram = nc.dram_tensor("p", [P, 2, DP1], F32R, kind="ExternalInput").ap()
    negc_dram = nc.dram_tensor("negc", [P, 1], F32, kind="ExternalInput").ap()
    ident_dram = nc.dram_tensor("ident", [P, P], F32R, kind="ExternalInput").ap()
    out_dram = nc.dram_tensor("out", [NS], F32, kind="ExternalOutput").ap()

    with tile.TileContext(nc) as tc, ExitStack() as ctx:
        singles = ctx.enter_context(tc.tile_pool(name="singles", bufs=1))
        xpool = ctx.enter_context(tc.tile_pool(name="xpool", bufs=6))
        xtpool = ctx.enter_context(tc.tile_pool(name="xtpool", bufs=4))
        wpool = ctx.enter_context(tc.tile_pool(name="wpool", bufs=4))
        psum_xt = ctx.enter_context(tc.tile_pool(name="psum_xt", bufs=2, space="PSUM"))
        psum_z = ctx.enter_context(tc.tile_pool(name="psum_z", bufs=2, space="PSUM"))

        # small preamble loads on the ACT HWDGE ring so they don't serialize
        # ahead of the first X loads on the SP ring
        ident = singles.tile([P, P], F32R)
        nc.scalar.dma_start(ident, ident_dram)
        pp = singles.tile([P, 2, DP1], F32R)
        nc.scalar.dma_start(pp, p_dram)
        negc = singles.tile([P, 1], F32)
        nc.scalar.dma_start(negc, negc_dram)
        # warm the ACT function table (~2.7us) off the critical path
        act_warm = singles.tile([P, 1], F32)
        nc.scalar.activation(
            act_warm,
            negc,
            mybir.ActivationFunctionType.Copy,
            scale=1.0,
            accum_out=None,
        )

        # two half-staging tiles so the mid-kernel flush of half 0 creates no
        # WAR dependency against the second half's reduce writes
        staging0 = singles.tile([P, TILES // 2], F32)
        staging1 = singles.tile([P, TILES // 2], F32)
        stagings = [staging0, staging1]

        def stage_col(t):
            h, off = divmod(t, TILES // 2)
            return stagings[h][:, off : off + 1]

        x_view = x_dram.rearrange("(t p) d -> p t d", p=P)  # [128, 64, 258]
        out_view = out_dram.rearrange("(t p) -> t p", p=P)
        H = TILES // 2

        def flush_half(h):
            # out[128*t + p] = staging[p, t]: transpose then contiguous DMA.
            # The final copy adds -c (reduces produced -sum = c - y^T P y).
            # borrows an xt_ps slot (same tag) — saves a PSUM bank
            st_ps = psum_xt.tile([H, P], F32, tag="xt_ps")
            nc.tensor.transpose(st_ps, stagings[h], ident.bitcast(F32))
            out_sb = singles.tile([H, P], F32, tag=f"out_sb{h}")
            # NOTE: must stay on ACT — DVE tensor_scalar with an AP scalar
            # lowers to InstTensorScalarPtr, which crashes this runtime
            nc.scalar.activation(
                out_sb,
                st_ps,
                mybir.ActivationFunctionType.Identity,
                bias=negc[0:H, 0:1],
                scale=1.0,
            )
            nc.sync.dma_start(out_view[h * H : (h + 1) * H, :], out_sb)

        for g in range(TILES // DMA_BATCH):
            if g == 0:
                # split the first group into pair DMAs so compute starts
                # after 256KB instead of 512KB (trims the startup stall)
                xg0 = []
                for j in range(2):
                    xb = xpool.tile([P, 2, DP1], F32R, tag="xg0")
                    nc.sync.dma_start(xb, x_view[:, 2 * j : 2 * j + 2, :])
                    xg0.append(xb)
                xpair = lambda j: xg0[j]
            else:
                xg = xpool.tile([P, DMA_BATCH, DP1], F32R)
                nc.sync.dma_start(
                    xg, x_view[:, g * DMA_BATCH : (g + 1) * DMA_BATCH, :]
                )
                xpair = lambda j: xg[:, 2 * j : 2 * j + 2, :]
            # transpose all 4 row-tiles into one 2-bank PSUM tile, then one
            # wide ACT copy (FD=1024) amortizes the ~370-cycle fixed overhead
            xt_ps = psum_xt.tile([P, 2 * DMA_BATCH, P], F32R)
            for b in range(DMA_BATCH):
                xr = xpair(b // 2)[:, b % 2, :]
                nc.tensor.transpose(xt_ps[:, 2 * b, :], xr[:, 0:P], ident)
                nc.tensor.transpose(xt_ps[:, 2 * b + 1, :], xr[:, P:D], ident)
            xt_sb = xtpool.tile([P, 2 * DMA_BATCH, P], F32R)
            if g == 0:
                # two half-copies so the first matmuls start after 2 tiles
                nc.scalar.copy(xt_sb[:, 0:4, :], xt_ps[:, 0:4, :])
                nc.scalar.copy(xt_sb[:, 4:8, :], xt_ps[:, 4:8, :])
            else:
                nc.scalar.copy(xt_sb, xt_ps)
            for j in range(2):  # process row-tiles in pairs
                pair = 2 * g + j
                t = 2 * pair
                # two tiles' Z side by side in one 2-bank PSUM tile
                z2 = psum_z.tile([P, 2, 512], F32)
                for b2 in range(2):
                    for k in range(2):
                        nc.tensor.matmul(
                            z2[:, b2, 0:DP1],
                            lhsT=xt_sb[:, 2 * (2 * j + b2) + k, :],
                            rhs=pp[:, k, :],
                            start=(k == 0),
                            stop=(k == 1),
                        )
                # W = Z~ * X~ for the pair in ONE DVE op (FD=516); reduces
                # split DVE/ACT to balance (fused reduce ops crash this rt)
                w2 = wpool.tile([P, 2, DP1], F32)
                if pair == 0:
                    # two single mults: DVE starts after 2 matmuls, not 4
                    for b2 in range(2):
                        nc.vector.tensor_mul(
                            w2[:, b2, :],
                            z2[:, b2, 0:DP1],
                            xpair(j)[:, b2, :].bitcast(F32),
                        )
                else:
                    nc.vector.tensor_mul(
                        w2, z2[:, :, 0:DP1], xpair(j).bitcast(F32)
                    )
                # interleave ACT pairs (3 of 8) among DVE pairs (5 of 8) so
                # DVE load is smooth and the z2 pool never backs up
                if pair % 8 not in (0, 3, 6):
                    h, off = divmod(t, H)
                    nc.vector.tensor_reduce(
                        stagings[h][:, off : off + 2],
                        w2,
                        axis=mybir.AxisListType.X,
                        op=ADD,
                        negate=True,
                    )
                else:
                    for b2 in range(2):
                        nc.scalar.activation(
                            w2[:, b2, :],
                            w2[:, b2, :],
                            mybir.ActivationFunctionType.Copy,
                            scale=-1.0,
                            accum_out=stage_col(t + b2),
                        )
                if pair == TILES // 4 + 2:
                    # staging0 completed 2 pairs ago → the flush transpose
                    # enters the in-order PE queue with no pending wait
                    flush_half(0)

        flush_half(1)

    nc.compile()

    return nc


def _get_program():
    global _PROGRAM
    if _PROGRAM is None:
        _PROGRAM = _build_program()
    return _PROGRAM


def _host_inputs(X, mean, prec):
    X = np.asarray(X, dtype=np.float32)
    X_pad = np.empty((N, DP1), dtype=np.float32)
    # pre-round X to fp32r (11-bit mantissa): fp32r-mode PE reads truncate
    xb = X.view(np.uint32)
    X_pad[:, :D].view(np.uint32)[:] = (xb + 0x800) & np.uint32(0xFFFFF000)
    X_pad[:, D] = 1.0
    X_pad[:, D + 1] = 0.0
    m = np.asarray(mean, dtype=np.float32).reshape(-1)
    Pm = np.asarray(prec, dtype=np.float32)
    u = (Pm + Pm.T) @ m
    c = float(m @ (Pm @ m))
    p_aug = np.concatenate(
        [Pm, -u[:, None], np.zeros((D, 1), np.float32)], axis=1
    )  # [256, 258]
    # pre-round to fp32r (fp32 with 11-bit mantissa, round-half-up on 12 LSBs)
    bits = p_aug.view(np.uint32)
    p_aug = (((bits + 0x800) & np.uint32(0xFFFFF000)).astype(np.uint32)).view(
        np.float32
    )
    p_host = np.ascontiguousarray(
        p_aug.reshape(2, P, DP1).transpose(1, 0, 2)
    )  # [128, 2, 258]
    negc_host = np.full((P, 1), -c, dtype=np.float32)
    ident_host = np.eye(P, dtype=np.float32)
    in_maps = [
        {
            "x": X_pad[i * NS : (i + 1) * NS],
            "p": p_host,
            "negc": negc_host,
            "ident": ident_host,
        }
        for i in range(N_CORES)
    ]
    return in_maps


def kernel(X, mean, prec):
    global LAST_EXEC_NS, LAST_RESULTS
    from concourse.bass_utils import run_bass_kernel_spmd

    nc = _get_program()
    in_maps = _host_inputs(X, mean, prec)
    res = run_bass_kernel_spmd(
        nc, in_maps, core_ids=list(range(N_CORES)), trace=TRACE
    )
    LAST_RESULTS = res
    LAST_EXEC_NS = res.exec_time_ns
    out = np.concatenate([res.results[i]["out"] for i in range(N_CORES)])
    return out.astype(np.float32)
